# revision 3
# baseline (speedup 1.0000x reference)
"""AUGRU cell (attention-scaled GRU update) on 8 Trainium2 NeuronCores.

Data-parallel: batch B=65536 is sharded 8 ways (8192 rows/core); the small
gate weights are replicated.  Per core:

  gates_x = x @ W_x.T + b_x          (8192,384)
  gates_h = h @ W_h.T + b_h
  u = sigmoid(.. u block ..); r = sigmoid(.. r block ..)
  h_tilde = tanh(x_c + r * h_c)
  h_new = h_prev + att*u*(h_tilde - h_prev)

Kernel layout (batch on partitions, gates along free dim):
  - 512-row groups (4 tiles of 128 rows).
  - x/h tiles PE-transposed ([I,128] stationaries), gates accumulated in
    PSUM banks laid out [Cx | S_u | S_r | Ch] (512 f32 = 1 bank/tile).
  - bias enters PSUM via a K=1 ones-matmul (start=True zero-fills the bank).
  - epilogue: ACT does sigmoid/sigmoid/tanh, DVE does the candidate muls and
    the fused (d*att)*u, GPSIMD does the two adds with h_prev.
"""

import sys

sys.path.insert(0, "/opt/trn_rl_repo")

from contextlib import ExitStack

import numpy as np

import concourse.bass as bass
import concourse.tile as tile
from concourse import bacc, mybir
from concourse.bass_utils import run_bass_kernel_spmd
from concourse.masks import make_identity

F32 = mybir.dt.float32
F32R = mybir.dt.float32r
AF = mybir.ActivationFunctionType
OP = mybir.AluOpType

B = 65536
NCORES = 8
BL = B // NCORES  # 8192 rows per core
I = 128
H = 128
G3 = 3 * H  # 384
P = 128  # partitions / tile rows
GROUP = 4  # tiles per group
ROWS = P * GROUP  # 512 rows per group
NGROUPS = BL // ROWS  # 16

# PSUM bank layout per batch-tile (512 f32 = one 2KB bank):
#   [0:128]   Cx' = x_c + b_xc
#   [128:256] S_u = x_u + h_u + b_xu + b_hu
#   [256:384] S_r = x_r + h_r + b_xr + b_hr
#   [384:512] Ch' = h_c + b_hc


def _r(ap):
    return ap.bitcast(F32R)


def build_program():
    nc = bacc.Bacc("TRN2", target_bir_lowering=False, debug=False)

    x_d = nc.dram_tensor("x", [BL, I], F32, kind="ExternalInput").ap()
    h_d = nc.dram_tensor("h_prev", [BL, H], F32, kind="ExternalInput").ap()
    a_d = nc.dram_tensor("att_score", [BL], F32, kind="ExternalInput").ap()
    wx_d = nc.dram_tensor("W_x", [G3, I], F32, kind="ExternalInput").ap()
    bx_d = nc.dram_tensor("b_x", [G3], F32, kind="ExternalInput").ap()
    wh_d = nc.dram_tensor("W_h", [G3, H], F32, kind="ExternalInput").ap()
    bh_d = nc.dram_tensor("b_h", [G3], F32, kind="ExternalInput").ap()
    o_d = nc.dram_tensor("h_new", [BL, H], F32, kind="ExternalOutput").ap()

    with tile.TileContext(nc) as tc, ExitStack() as ctx:
        consts = ctx.enter_context(tc.tile_pool(name="consts", bufs=1))
        io = ctx.enter_context(tc.tile_pool(name="io", bufs=3))
        tr = ctx.enter_context(tc.tile_pool(name="tr", bufs=3))
        ep = ctx.enter_context(tc.tile_pool(name="ep", bufs=2))
        pt = ctx.enter_context(
            tc.tile_pool(name="pt", bufs=2, space="PSUM")
        )  # transposes: [128,2,512] = 2 banks/buf
        pg = ctx.enter_context(
            tc.tile_pool(name="pg", bufs=1, space="PSUM")
        )  # gates: [128,4,512] = 4 banks

        # ---------------- one-time setup ----------------
        ident = consts.tile([P, P], F32)
        make_identity(nc, ident)
        ones_f = consts.tile([1, P], F32, tag="ones_f")
        nc.vector.memset(ones_f, 1.0)
        ones = consts.tile([1, P], F32R)
        nc.vector.tensor_copy(ones, ones_f)

        # Weight blocks arrive [384,128] row-major; load as [gate128, block, I]
        wxn = consts.tile([P, 3, I], F32, tag="wxn")
        nc.sync.dma_start(wxn, wx_d.rearrange("(b g) i -> g b i", g=P))
        whn = consts.tile([P, 3, I], F32, tag="whn")
        nc.sync.dma_start(whn, wh_d.rearrange("(b g) i -> g b i", g=P))

        # Transposed weights, reordered:
        #   wtx columns [c|u|r]  (x matmul writes bank cols [0:384])
        #   wth columns [u|r|c]  (h matmul writes bank cols [128:512])
        wtx = consts.tile([P, G3], F32R, tag="wtx")
        wth = consts.tile([P, G3], F32R, tag="wth")
        for dst, src in ((0, 2), (1, 0), (2, 1)):
            ps = pt.tile([P, 2, ROWS], F32, tag="xh")
            nc.tensor.matmul(
                ps[:, 0, 0:P], lhsT=wxn[:, src, :], rhs=ident,
                is_transpose=True,
            )
            nc.vector.tensor_copy(wtx[:, dst * P : (dst + 1) * P], ps[:, 0, 0:P])
        for dst in range(3):
            ps = pt.tile([P, 2, ROWS], F32, tag="xh")
            nc.tensor.matmul(
                ps[:, 0, 0:P], lhsT=whn[:, dst, :], rhs=ident,
                is_transpose=True,
            )
            nc.vector.tensor_copy(wth[:, dst * P : (dst + 1) * P], ps[:, 0, 0:P])

        # att scores: load [64,128] (contig per partition), transpose to [128,64]
        att_n = consts.tile([BL // P, P], F32, tag="attn")
        nc.sync.dma_start(att_n, a_d.rearrange("(j p) -> j p", p=P))
        att = consts.tile([P, BL // P], F32, tag="att")
        ps = pt.tile([P, 2, ROWS], F32, tag="xh")
        nc.tensor.matmul(
            ps[:, 0, 0 : BL // P], lhsT=att_n, rhs=ident[0 : BL // P, 0 : BL // P],
            is_transpose=True,
        )
        nc.vector.tensor_copy(att, ps[:, 0, 0 : BL // P])

        # combined bias vector [Cx | u | r | Ch] on partition 0
        bxs = consts.tile([1, G3], F32, tag="bxs")
        nc.sync.dma_start(bxs, bx_d.unsqueeze(0))
        bhs = consts.tile([1, G3], F32, tag="bhs")
        nc.sync.dma_start(bhs, bh_d.unsqueeze(0))
        bias = consts.tile([1, 4 * P], F32R, tag="bias")
        nc.vector.tensor_copy(bias[:, 0:128], bxs[:, 256:384])
        nc.vector.tensor_tensor(bias[:, 128:384], bxs[:, 0:256], bhs[:, 0:256], OP.add)
        nc.vector.tensor_copy(bias[:, 384:512], bhs[:, 256:384])

        # ---------------- main loop ----------------
        for g in range(NGROUPS):
            b0 = g * ROWS
            xn = io.tile([P, GROUP, I], F32, tag="xn")
            nc.sync.dma_start(xn, x_d[b0 : b0 + ROWS, :].rearrange("(t p) i -> p t i", p=P))
            hn = io.tile([P, GROUP, H], F32, tag="hn")
            nc.sync.dma_start(hn, h_d[b0 : b0 + ROWS, :].rearrange("(t p) i -> p t i", p=P))

            # PE transposes: x tiles -> bank 0, h tiles -> bank 1
            tps = pt.tile([P, 2, ROWS], F32, tag="xh")
            for t in range(GROUP):
                nc.tensor.matmul(
                    tps[:, 0, t * P : (t + 1) * P], lhsT=xn[:, t, :], rhs=ident,
                    is_transpose=True, start=(t == 0), stop=(t == GROUP - 1),
                )
            for t in range(GROUP):
                nc.tensor.matmul(
                    tps[:, 1, t * P : (t + 1) * P], lhsT=hn[:, t, :], rhs=ident,
                    is_transpose=True, start=(t == 0), stop=(t == GROUP - 1),
                )
            xT = tr.tile([P, ROWS], F32R, tag="xT")
            nc.vector.tensor_copy(xT, tps[:, 0, :])
            hT = tr.tile([P, ROWS], F32R, tag="hT")
            nc.scalar.copy(hT, tps[:, 1, :])

            # gates: 3 matmuls per tile into one PSUM bank each
            gp = pg.tile([P, GROUP, 4 * P], F32, tag="g")
            for t in range(GROUP):
                nc.tensor.matmul(
                    gp[:, t, :], lhsT=ones, rhs=bias, start=True, stop=False
                )
                nc.tensor.matmul(
                    gp[:, t, 0:G3], lhsT=xT[:, t * P : (t + 1) * P], rhs=wtx,
                    start=False, stop=False,
                )
                nc.tensor.matmul(
                    gp[:, t, P : P + G3], lhsT=hT[:, t * P : (t + 1) * P], rhs=wth,
                    start=False, stop=True,
                )

            # epilogue (all [128, 4, 128] views)
            u = ep.tile([P, GROUP, H], F32, tag="u")
            nc.scalar.activation(u, gp[:, :, 128:256], AF.Sigmoid)
            r = ep.tile([P, GROUP, H], F32, tag="r")
            nc.scalar.activation(r, gp[:, :, 256:384], AF.Sigmoid)
            m = ep.tile([P, GROUP, H], F32, tag="m")
            nc.vector.tensor_tensor(m, r, gp[:, :, 384:512], OP.mult)
            pre = ep.tile([P, GROUP, H], F32, tag="pre")
            nc.vector.tensor_tensor(pre, m, gp[:, :, 0:128], OP.add)
            th = ep.tile([P, GROUP, H], F32, tag="th")
            nc.scalar.activation(th, pre, AF.Tanh)
            d = ep.tile([P, GROUP, H], F32, tag="d")
            nc.gpsimd.tensor_tensor(d, th, hn, OP.subtract)
            e = ep.tile([P, GROUP, H], F32, tag="e")
            for t in range(GROUP):
                nc.vector.scalar_tensor_tensor(
                    e[:, t, :], in0=d[:, t, :],
                    scalar=att[:, g * GROUP + t : g * GROUP + t + 1],
                    in1=u[:, t, :], op0=OP.mult, op1=OP.mult,
                )
            ho = ep.tile([P, GROUP, H], F32, tag="ho")
            nc.gpsimd.tensor_tensor(ho, e, hn, OP.add)
            nc.sync.dma_start(
                o_d[b0 : b0 + ROWS, :].rearrange("(t p) i -> p t i", p=P), ho
            )

    nc.compile()
    return nc


_NC_CACHE = []


def _get_nc():
    if not _NC_CACHE:
        _NC_CACHE.append(build_program())
    return _NC_CACHE[0]


def kernel(x, h_prev, att_score, W_x, b_x, W_h, b_h, **_unused):
    x = np.ascontiguousarray(np.asarray(x, dtype=np.float32))
    h_prev = np.ascontiguousarray(np.asarray(h_prev, dtype=np.float32))
    att_score = np.ascontiguousarray(np.asarray(att_score, dtype=np.float32))
    W_x = np.ascontiguousarray(np.asarray(W_x, dtype=np.float32))
    b_x = np.ascontiguousarray(np.asarray(b_x, dtype=np.float32))
    W_h = np.ascontiguousarray(np.asarray(W_h, dtype=np.float32))
    b_h = np.ascontiguousarray(np.asarray(b_h, dtype=np.float32))

    nc = _get_nc()
    in_maps = []
    for c in range(NCORES):
        s = slice(c * BL, (c + 1) * BL)
        in_maps.append(
            {
                "x": np.ascontiguousarray(x[s]),
                "h_prev": np.ascontiguousarray(h_prev[s]),
                "att_score": np.ascontiguousarray(att_score[s]),
                "W_x": W_x,
                "b_x": b_x,
                "W_h": W_h,
                "b_h": b_h,
            }
        )
    res = run_bass_kernel_spmd(nc, in_maps, list(range(NCORES)))
    out = np.concatenate([res.results[c]["h_new"] for c in range(NCORES)], axis=0)
    return out
